# revision 2
# baseline (speedup 1.0000x reference)
"""Trainium2 Bass kernel: fused store_kvcache + causal prefill attention.

Problem (hardcoded): T=8192 tokens, H=16 heads, D=128, seq_len=2048 (B=4
packed sequences), fp32 in/out. slot_mapping is arange(T) (contiguous slots),
so the KV-cache scatter followed by the cache gather is an identity
permutation on [0,T): attention reads exactly k/v. For robustness, any
non-identity slot_mapping is materialized on the host before the device call.

Sharding: tensor-parallel over heads. 16 heads / 8 NeuronCores = 2 heads per
core; each core runs the same Bass program on its own head slice (SPMD).
Host-side prep per core: slice the 2 heads and lay Q/K out d-major
([head, batch, d, token]) in bf16 - the layout the PE contraction needs.

Per (batch, head) the device computes, per 512-query block (bf16 matmul
operands, fp32 PSUM accumulation), with work batched into large "groups"
so the scalar engine (the bottleneck: exp at 1 elem/cycle/lane + ~293ns
per-instruction overhead) runs few, large ACTIVATEs:
  - off-diagonal k-tiles are grouped 3 per PSUM tile [128, 3, 512] (3 banks);
    one QK matmul per k-tile, ONE exp per group.
  - the 4 diagonal k-tiles of each query block form one exact-causal PSUM
    tile [128, 1280] (segment order mi0,mi1,mi3,mi2 keeps every matmul
    output inside a single 2KB PSUM bank); ONE exp per group; the
    lower-triangle masking (DVE multiply) only touches 2x[128,2,128] +
    [128,256] column ranges.
  - softmax denominator partials accumulate into two bf16 halves on the DVE
    (copy-init on first touch, [128,2,512] paired adds when possible); the
    host does the final 256-way sum + divide while gathering/transposing.
  - PV accumulates O^T into a 1-bank fp32 PSUM tile per query block
    (per-element has_written accumulation, exact-causal moving ranges).
PSUM budget: 2 x 3-bank score tiles + 2 x 1-bank O^T tiles = 8 banks.
"""

import numpy as np
import ml_dtypes

import concourse.bacc as bacc
import concourse.tile as tile
from concourse import mybir
from concourse.bass_utils import run_bass_kernel_spmd

# Problem constants (match the grading harness inputs).
T, H, D = 8192, 16, 128
SEQ_LEN = 2048
NUM_SLOTS = 16384
SCALE = 0.08838834764831845  # 1/sqrt(128)
N_CORES = 8
HPC = H // N_CORES  # heads per core
B = T // SEQ_LEN

BF16 = mybir.dt.bfloat16
F32 = mybir.dt.float32

QBLK = 512           # query block (one PSUM bank of fp32 for O^T)
NMI = QBLK // 128    # 128-chunks per query block

# Diagonal group segments: (mi, elem offset in the [128, 1280] PSUM tile,
# moving width N). Order mi0,mi1,mi3,mi2 so no matmul output crosses a 2KB
# PSUM bank boundary (bank splits at elem 512 and 1024 in fp32).
DIAG_SEGS = [(0, 0, 512), (1, 512, 384), (3, 896, 128), (2, 1024, 256)]
DIAG_ELEMS = 1280


def _build_groups(nblk):
    """Per-(b,h) ACT-group schedule. Each group is (kind, segs):
      kind "off": segs = [(blk, j), ...] off-diagonal k-tiles, <=3 per group
                  (3 PSUM banks), may span two query blocks.
      kind "diag": segs = blk; the 4 diagonal tiles of query block blk.
    Order guarantees: all off-diag segs of blk appear before diag(blk); the
    first seg of each blk is full-width (N=512)."""
    assert nblk == 4
    g = [
        ("diag", 0),
        ("off", [(1, 0), (1, 1), (1, 2)]),
        ("off", [(1, 3), (2, 0), (2, 1)]),
        ("diag", 1),
        ("off", [(2, 2), (2, 3), (2, 4)]),
        ("off", [(2, 5), (2, 6), (2, 7)]),
        ("diag", 2),
        ("off", [(3, 0), (3, 1), (3, 2)]),
        ("off", [(3, 3), (3, 4), (3, 5)]),
        ("off", [(3, 6), (3, 7), (3, 8)]),
        ("off", [(3, 9), (3, 10), (3, 11)]),
        ("diag", 3),
    ]
    return g


# segs per query block: off-diag tiles + 4 diag segs
SEGS_PER_BLK = [4, 8, 12, 16]


def build_attention(nc, qT_d, kT_d, vh, masks, oh, ah, S, B_, HPC_):
    """Emit the Tile program.

    qT_d/kT_d: DRAM APs [HPC_, B_, 128, S] bf16 (d-major Q/K).
    vh:        DRAM AP [B_*S, HPC_, 128] fp32 (natural V).
    masks:     DRAM AP [128, 256] bf16 ([tri | tri] causal masks).
    oh:        DRAM AP [HPC_, B_, NBLK, 128, QBLK] bf16 output: UNNORMALIZED
               O^T blocks (host divides by denominators and transposes back).
    ah:        DRAM AP [HPC_, B_, NBLK, 128, 2, QBLK] bf16 output: softmax
               denominator accumulator halves (host sums across the 128x2).
    """
    NT = S // 128           # 128-token tiles per sequence
    NBLK = S // QBLK        # query blocks per sequence

    with tile.TileContext(nc) as tc:
        with (
            tc.tile_pool(name="singles", bufs=1) as singles,
            tc.tile_pool(name="dmaj", bufs=2) as dmaj,
            tc.tile_pool(name="ptp", bufs=4) as ptp,
            tc.tile_pool(name="accp", bufs=3) as accp,
            tc.tile_pool(name="outp", bufs=4) as outp,
            tc.tile_pool(name="ps_s", bufs=2, space="PSUM") as ps_s,
            tc.tile_pool(name="ps_o", bufs=2, space="PSUM") as ps_o,
        ):
            tri2 = singles.tile([128, 256], BF16)
            nc.sync.dma_start(out=tri2, in_=masks)

            for b in range(B_):
                for h in range(HPC_):
                    base = b * S
                    # d-major Q/K: straight HWDGE loads, contiguous 4KB rows
                    qT = dmaj.tile([128, NT, 128], BF16, tag="qT")
                    nc.gpsimd.dma_start(
                        out=qT, in_=qT_d[h, b].rearrange("d (n p) -> d n p", p=128)
                    )
                    kT = dmaj.tile([128, NT, 128], BF16, tag="kT")
                    nc.gpsimd.dma_start(
                        out=kT, in_=kT_d[h, b].rearrange("d (n p) -> d n p", p=128)
                    )
                    # natural V tiles, fp32->bf16 cast in the SWDGE datapath
                    vsrc = vh[base : base + S, h, :].rearrange(
                        "(n p) d -> p n d", p=128
                    )
                    vsb = dmaj.tile([128, NT, 128], BF16, tag="vsb")
                    nc.gpsimd.dma_start(out=vsb, in_=vsrc)

                    groups = _build_groups(NBLK)
                    ctx = {}
                    sbuf = {}   # group idx -> PSUM score tile
                    pbuf = {}   # group idx -> SBUF bf16 P tile

                    def get_ctx(blk):
                        if blk not in ctx:
                            o_ps = ps_o.tile([128, QBLK], F32, tag="o_ps")
                            acc2 = accp.tile([128, 2, QBLK], BF16, tag="acc2")
                            ctx[blk] = {
                                "o": o_ps, "a": acc2,
                                "init": [False, False],  # per-half acc init
                                "c": 0,                  # seg counter (half toggle)
                                "pv_first": True,
                                "left": SEGS_PER_BLK[blk],
                            }
                        return ctx[blk]

                    def emit_qk(u):
                        kind, segs = groups[u]
                        if kind == "off":
                            s = ps_s.tile([128, len(segs), 512], F32, tag="s3")
                            for i, (blk, j) in enumerate(segs):
                                qmov = qT[:, blk * NMI : (blk + 1) * NMI, :]
                                nc.tensor.matmul(
                                    s[:, i, :], lhsT=kT[:, j, :], rhs=qmov,
                                    start=True, stop=True,
                                )
                        else:
                            blk = segs
                            s = ps_s.tile([128, DIAG_ELEMS], F32, tag="s3")
                            for mi, off, n in DIAG_SEGS:
                                nch = n // 128
                                qmov = qT[:, (blk + 1) * NMI - nch : (blk + 1) * NMI, :]
                                nc.tensor.matmul(
                                    s[:, off : off + n],
                                    lhsT=kT[:, blk * NMI + mi, :], rhs=qmov,
                                    start=True, stop=True,
                                )
                        sbuf[u] = s

                    def emit_exp(u):
                        kind, segs = groups[u]
                        s = sbuf.pop(u)
                        if kind == "off":
                            pT = ptp.tile([128, len(segs), 512], BF16, tag="pT")
                        else:
                            pT = ptp.tile([128, DIAG_ELEMS], BF16, tag="pT")
                        nc.scalar.activation(
                            out=pT, in_=s,
                            func=mybir.ActivationFunctionType.Exp, scale=SCALE,
                        )
                        pbuf[u] = pT

                    def acc_add(cx, src, half, lo=0):
                        """acc2[:, half, lo:] (+)= src  (copy on first touch)."""
                        acc2 = cx["a"]
                        dst = acc2[:, half, lo:]
                        if cx["init"][half]:
                            nc.vector.tensor_add(dst, dst, src)
                        else:
                            if lo:
                                nc.vector.memset(acc2[:, half, 0:lo], 0.0)
                            nc.vector.tensor_copy(dst, src)
                            cx["init"][half] = True

                    def acc_pair(cx, src2):
                        """acc2[:, 0:2, :] (+)= src2 ([128,2,512], halves 0,1)."""
                        acc2 = cx["a"]
                        dst = acc2[:, 0:2, :]
                        if cx["init"][0] and cx["init"][1]:
                            nc.vector.tensor_add(dst, dst, src2)
                        else:
                            assert not cx["init"][0] and not cx["init"][1]
                            nc.vector.tensor_copy(dst, src2)
                            cx["init"][0] = cx["init"][1] = True

                    def pv(cx, vtile, rhs, lo=0):
                        o_ps = cx["o"]
                        cx["left"] -= 1
                        nc.tensor.matmul(
                            o_ps[:, lo:] if lo else o_ps,
                            lhsT=vtile, rhs=rhs,
                            start=cx["pv_first"], stop=(cx["left"] == 0),
                            skip_group_check=True,
                        )
                        cx["pv_first"] = False

                    def close_blk(b_, h_, blk, cx):
                        oT_sb = outp.tile([128, QBLK], BF16, tag="oT_sb")
                        nc.vector.tensor_copy(oT_sb, cx["o"])
                        nc.sync.dma_start(out=oh[h_, b_, blk], in_=oT_sb)
                        nc.sync.dma_start(out=ah[h_, b_, blk], in_=cx["a"])
                        del ctx[blk]

                    def emit_rest(u, b_, h_):
                        kind, segs = groups[u]
                        pT = pbuf.pop(u)
                        if kind == "off":
                            i = 0
                            while i < len(segs):
                                blk, j = segs[i]
                                cx = get_ctx(blk)
                                half = cx["c"] % 2
                                # pair two same-blk consecutive tiles on (h0,h1)
                                if (half == 0 and i + 1 < len(segs)
                                        and segs[i + 1][0] == blk):
                                    acc_pair(cx, pT[:, i : i + 2, :])
                                    cx["c"] += 2
                                    pv(cx, vsb[:, j, :], pT[:, i, :])
                                    pv(cx, vsb[:, segs[i + 1][1], :], pT[:, i + 1, :])
                                    i += 2
                                else:
                                    acc_add(cx, pT[:, i, :], half)
                                    cx["c"] += 1
                                    pv(cx, vsb[:, j, :], pT[:, i, :])
                                    i += 1
                        else:
                            blk = segs
                            cx = get_ctx(blk)
                            # causal masks on the first 128 columns of each
                            # segment: {mi0@0, mi1@512} strided, {mi3@896,
                            # mi2@1024} contiguous
                            v01 = pT[:, 0:1024].rearrange(
                                "p (a c) -> p a c", c=512
                            )[:, :, 0:128]
                            t2 = tri2.rearrange("p (a c) -> p a c", c=128)
                            nc.vector.tensor_mul(v01, v01, t2)
                            v23 = pT[:, 896:1152]
                            nc.vector.tensor_mul(v23, v23, tri2)
                            for mi, off, n in DIAG_SEGS:
                                half = cx["c"] % 2
                                acc_add(cx, pT[:, off : off + n], half,
                                        lo=QBLK - n)
                                cx["c"] += 1
                                pv(cx, vsb[:, blk * NMI + mi, :],
                                   pT[:, off : off + n], lo=QBLK - n)
                            close_blk(b_, h_, blk, cx)

                    n_u = len(groups)
                    for u in range(n_u):
                        emit_qk(u)
                        if u >= 1:
                            emit_exp(u - 1)
                        if u >= 2:
                            emit_rest(u - 2, b, h)
                    emit_exp(n_u - 1)
                    emit_rest(n_u - 2, b, h)
                    emit_rest(n_u - 1, b, h)


def build_masks():
    """[tri | tri]: lower-triangular (inclusive) causal keep-mask for the
    first 128 columns of a diagonal segment, duplicated so one [128, 256]
    tensor serves both the strided {mi0,mi1} and contiguous {mi3,mi2}
    multiplies."""
    p = np.arange(128)[:, None]
    y = np.arange(128)[None, :]
    tri = (y >= p)
    return np.concatenate([tri, tri], axis=1).astype(ml_dtypes.bfloat16)


_CACHED = {}


def _get_program():
    if "nc" not in _CACHED:
        nc = bacc.Bacc("TRN2", target_bir_lowering=False)
        qT_d = nc.dram_tensor(
            "qTh", [HPC, B, D, SEQ_LEN], BF16, kind="ExternalInput"
        ).ap()
        kT_d = nc.dram_tensor(
            "kTh", [HPC, B, D, SEQ_LEN], BF16, kind="ExternalInput"
        ).ap()
        vh = nc.dram_tensor("vh", [T, HPC, D], F32, kind="ExternalInput").ap()
        masks = nc.dram_tensor(
            "masks", [128, 256], BF16, kind="ExternalInput"
        ).ap()
        oh = nc.dram_tensor(
            "oh", [HPC, B, SEQ_LEN // QBLK, D, QBLK], BF16,
            kind="ExternalOutput",
        ).ap()
        ah = nc.dram_tensor(
            "ah", [HPC, B, SEQ_LEN // QBLK, 128, 2, QBLK], BF16,
            kind="ExternalOutput",
        ).ap()
        build_attention(nc, qT_d, kT_d, vh, masks, oh, ah, SEQ_LEN, B, HPC)
        nc.compile()  # bacc passes: split >1-wait syncs into event semaphores
        _CACHED["nc"] = nc
    return _CACHED["nc"]


def _host_resolve_kv(k, v, k_cache, v_cache, slot_mapping):
    """Apply the cache scatter+gather on the host iff it is not the identity."""
    sm = np.asarray(slot_mapping)
    if sm.shape == (T,) and np.array_equal(sm, np.arange(T, dtype=sm.dtype)):
        return k, v
    kc = np.array(k_cache, dtype=np.float32, copy=True)
    vc = np.array(v_cache, dtype=np.float32, copy=True)
    valid = sm >= 0
    kc[sm[valid]] = k.reshape(T, H * D)[valid]
    vc[sm[valid]] = v.reshape(T, H * D)[valid]
    return kc[:T].reshape(T, H, D), vc[:T].reshape(T, H, D)


def _dmajor(x):
    """[T, H, D] fp32 -> [H, B, D, S] bf16 (d-major per sequence)."""
    xb = x.astype(ml_dtypes.bfloat16)
    return np.ascontiguousarray(
        xb.reshape(B, SEQ_LEN, H, D).transpose(2, 0, 3, 1)
    )


def kernel(q, k, v, k_cache, v_cache, slot_mapping, seq_len, _trace=False,
           _trace_kwargs=None):
    q = np.asarray(q, dtype=np.float32)
    k = np.asarray(k, dtype=np.float32)
    v = np.asarray(v, dtype=np.float32)
    assert q.shape == (T, H, D), q.shape
    assert int(seq_len) == SEQ_LEN, seq_len

    k, v = _host_resolve_kv(k, v, np.asarray(k_cache), np.asarray(v_cache),
                            slot_mapping)

    qTm = _dmajor(q)  # [H, B, D, S] bf16
    kTm = _dmajor(k)
    masks = build_masks()
    nc = _get_program()
    in_maps = []
    for c in range(N_CORES):
        hs = slice(c * HPC, (c + 1) * HPC)
        in_maps.append({
            "qTh": np.ascontiguousarray(qTm[hs]),
            "kTh": np.ascontiguousarray(kTm[hs]),
            "vh": np.ascontiguousarray(v[:, hs, :]),
            "masks": masks,
        })
    res = run_bass_kernel_spmd(
        nc, in_maps, core_ids=list(range(N_CORES)),
        trace=_trace, **(_trace_kwargs or {}),
    )
    out = np.empty((T, H, D), dtype=np.float32)
    for c in range(N_CORES):
        oT = np.asarray(res.results[c]["oh"]).astype(np.float32)
        av = np.asarray(res.results[c]["ah"]).astype(np.float32)
        denom = av.sum(axis=(3, 4))  # [HPC, B, NBLK, QBLK]
        o = oT / denom[:, :, :, None, :]
        # -> [B, NBLK, QBLK, HPC, D] -> [T, HPC, D]
        o = o.transpose(1, 2, 4, 0, 3).reshape(T, HPC, D)
        out[:, c * HPC : (c + 1) * HPC, :] = o
    if _trace:
        kernel.last_results = res
    return out


# revision 9
# speedup vs baseline: 1.0968x; 1.0968x over previous
"""Trainium2 Bass kernel: fused store_kvcache + causal prefill attention.

Problem (hardcoded): T=8192 tokens, H=16 heads, D=128, seq_len=2048 (B=4
packed sequences), fp32 in/out. slot_mapping is arange(T) (contiguous slots),
so the KV-cache scatter followed by the cache gather is an identity
permutation on [0,T): attention reads exactly k/v. For robustness, any
non-identity slot_mapping is materialized on the host before the device call.

Sharding: tensor-parallel over heads. 16 heads / 8 NeuronCores = 2 heads per
core; each core runs the same Bass program on its own head slice (SPMD).
Host-side prep per core: slice the 2 heads and lay Q/K out d-major
([head, batch, d, token]) in bf16 - the layout the PE contraction needs.

Per (batch, head) the device computes, per 512-query block (bf16 matmul
operands, fp32 PSUM accumulation), with the scalar engine (exp at 1
elem/cycle/lane) as the pacing engine:
  - off-diagonal k-tiles in pairs per PSUM tile [128, 2, 512]; one QK
    matmul per k-tile, ONE exp per pair.
  - the 4 diagonal k-tiles of each query block are exact-causal in two
    chunks [mi0 512|mi1 384] and [mi3 128|mi2 256] (segment offsets keep
    every matmul output inside a single 2KB PSUM bank); masking (DVE
    multiply) only touches the triangle columns.
  - all s-tiles are <= 2 banks with a 3-deep pool, so the PE runs 2+
    groups ahead of the ACT stream and every ACTIVATE's semaphore wait is
    pre-satisfied (keeps the ~290ns instruction setup overlapped).
  - softmax denominator partials accumulate into two bf16 halves on the DVE
    (copy-init on first touch, [128,2,512] paired adds when possible); the
    host does the final 256-way sum + divide while gathering/transposing.
  - PV accumulates O^T into a 1-bank fp32 PSUM tile per query block
    (per-element has_written accumulation, exact-causal moving ranges).
PSUM budget: 3 x 2-bank score tiles + 2 x 1-bank O^T tiles = 8 banks.
"""

import numpy as np
import ml_dtypes

import concourse.bacc as bacc
import concourse.tile as tile
from concourse import mybir
from concourse.bass_utils import run_bass_kernel_spmd

# Problem constants (match the grading harness inputs).
T, H, D = 8192, 16, 128
SEQ_LEN = 2048
NUM_SLOTS = 16384
SCALE = 0.08838834764831845  # 1/sqrt(128)
N_CORES = 8
HPC = H // N_CORES  # heads per core
B = T // SEQ_LEN

BF16 = mybir.dt.bfloat16
F32 = mybir.dt.float32

QBLK = 512           # query block (one PSUM bank of fp32 for O^T)
NMI = QBLK // 128    # 128-chunks per query block

# Diagonal tiles are processed exact-causal in two 2-bank-or-less chunks:
#   chunk "da": [mi0 N=512 | mi1 N=384]  (896 elems; mi1 starts at elem 512
#               = the 2KB PSUM bank boundary, so neither matmul output
#               crosses a bank)
#   chunk "db": [mi3 N=128 | mi2 N=256]  (384 elems, one bank)
# Each chunk's causal triangle sits in the first 128 columns of each
# segment; "da" masks {0, 512} (strided), "db" masks [0:256] (contiguous).
DA_SEGS = [(0, 0, 512), (1, 512, 384)]
DB_SEGS = [(3, 0, 128), (2, 128, 256)]


def _build_groups(nblk):
    """Per-(b,h) ACT-group schedule, one ACTIVATE per group, all s-tiles
    <= 2 PSUM banks (x3 pool bufs keeps the PE 2+ groups ahead of the
    scalar engine, preserving ACT instruction-setup overlap).
      ("off", blk, [j, j+1]): two off-diagonal k-tiles of query block blk.
      ("da"|"db", blk, segs): diagonal chunks (exact causal ranges).
    The first group touching blk is full-width; diag comes last."""
    assert nblk == 4
    g = [("da", 0, DA_SEGS), ("db", 0, DB_SEGS)]
    for blk in range(1, nblk):
        for j in range(0, 4 * blk, 2):
            g.append(("off", blk, [j, j + 1]))
        g.append(("da", blk, DA_SEGS))
        g.append(("db", blk, DB_SEGS))
    return g


# PV matmuls per query block: off-diag tiles + 4 diag segs
SEGS_PER_BLK = [4, 8, 12, 16]


def build_attention(nc, qT_d, kT_d, vh, masks, oh, ah, S, B_, HPC_):
    """Emit the Tile program.

    qT_d/kT_d: DRAM APs [HPC_, B_, 128, S] bf16 (d-major Q/K).
    vh:        DRAM AP [B_*S, HPC_, 128] fp32 (natural V).
    masks:     DRAM AP [128, 256] bf16 ([tri | tri] causal masks).
    oh:        DRAM AP [HPC_, B_, NBLK, 128, QBLK] bf16 output: UNNORMALIZED
               O^T blocks (host divides by denominators and transposes back).
    ah:        DRAM AP [HPC_, B_, NBLK, 128, 2, QBLK] bf16 output: softmax
               denominator accumulator halves (host sums across the 128x2).
    """
    NT = S // 128           # 128-token tiles per sequence
    NBLK = S // QBLK        # query blocks per sequence

    with tile.TileContext(nc) as tc:
        with (
            tc.tile_pool(name="singles", bufs=1) as singles,
            tc.tile_pool(name="dmaj", bufs=2) as dmaj,
            tc.tile_pool(name="ptp", bufs=4) as ptp,
            tc.tile_pool(name="accp", bufs=3) as accp,
            tc.tile_pool(name="outp", bufs=4) as outp,
            tc.tile_pool(name="ps_s", bufs=3, space="PSUM") as ps_s,
            tc.tile_pool(name="ps_o", bufs=2, space="PSUM") as ps_o,
        ):
            tri2 = singles.tile([128, 256], BF16)
            nc.sync.dma_start(out=tri2, in_=masks)

            for b in range(B_):
                for h in range(HPC_):
                    base = b * S
                    # d-major Q/K: straight HWDGE loads, contiguous 4KB rows
                    qT = dmaj.tile([128, NT, 128], BF16, tag="qT")
                    nc.gpsimd.dma_start(
                        out=qT, in_=qT_d[h, b].rearrange("d (n p) -> d n p", p=128)
                    )
                    kT = dmaj.tile([128, NT, 128], BF16, tag="kT")
                    nc.gpsimd.dma_start(
                        out=kT, in_=kT_d[h, b].rearrange("d (n p) -> d n p", p=128)
                    )
                    # natural V tiles, fp32->bf16 cast in the SWDGE datapath
                    vsrc = vh[base : base + S, h, :].rearrange(
                        "(n p) d -> p n d", p=128
                    )
                    vsb = dmaj.tile([128, NT, 128], BF16, tag="vsb")
                    nc.gpsimd.dma_start(out=vsb, in_=vsrc)

                    groups = _build_groups(NBLK)
                    ctx = {}
                    sbuf = {}   # group idx -> PSUM score tile
                    pbuf = {}   # group idx -> SBUF bf16 P tile

                    def get_ctx(blk):
                        if blk not in ctx:
                            o_ps = ps_o.tile([128, QBLK], F32, tag="o_ps")
                            acc2 = accp.tile([128, 2, QBLK], BF16, tag="acc2")
                            ctx[blk] = {
                                "o": o_ps, "a": acc2,
                                "init": [False, False],  # per-half acc init
                                "c": 0,                  # seg counter (half toggle)
                                "pv_first": True,
                                "left": SEGS_PER_BLK[blk],
                            }
                        return ctx[blk]

                    def emit_qk(u):
                        kind, blk, segs = groups[u]
                        if kind == "off":
                            s = ps_s.tile([128, 2, 512], F32, tag="s2")
                            qmov = qT[:, blk * NMI : (blk + 1) * NMI, :]
                            for i, j in enumerate(segs):
                                nc.tensor.matmul(
                                    s[:, i, :], lhsT=kT[:, j, :], rhs=qmov,
                                    start=True, stop=True,
                                )
                        else:
                            n_tot = 896 if kind == "da" else 384
                            s = ps_s.tile([128, n_tot], F32, tag="s2")
                            for mi, off, n in segs:
                                nch = n // 128
                                qmov = qT[:, (blk + 1) * NMI - nch : (blk + 1) * NMI, :]
                                nc.tensor.matmul(
                                    s[:, off : off + n],
                                    lhsT=kT[:, blk * NMI + mi, :], rhs=qmov,
                                    start=True, stop=True,
                                )
                        sbuf[u] = s

                    def emit_exp(u):
                        kind, blk, segs = groups[u]
                        s = sbuf.pop(u)
                        if kind == "off":
                            pT = ptp.tile([128, 2, 512], BF16, tag="pT")
                            dst = pT
                        elif kind == "da":
                            # padded to 1024 so the strided mask view
                            # ("p (a c) -> p a c", c=512) is constructible;
                            # [896:1024] is never written or read
                            pT = ptp.tile([128, 1024], BF16, tag="pT")
                            dst = pT[:, 0:896]
                        else:
                            pT = ptp.tile([128, 384], BF16, tag="pT")
                            dst = pT
                        nc.scalar.activation(
                            out=dst, in_=s,
                            func=mybir.ActivationFunctionType.Exp, scale=SCALE,
                        )
                        pbuf[u] = pT

                    def acc_add(cx, src, half, lo=0):
                        """acc2[:, half, lo:] (+)= src  (copy on first touch)."""
                        acc2 = cx["a"]
                        dst = acc2[:, half, lo:]
                        if cx["init"][half]:
                            nc.vector.tensor_add(dst, dst, src)
                        else:
                            if lo:
                                nc.vector.memset(acc2[:, half, 0:lo], 0.0)
                            nc.vector.tensor_copy(dst, src)
                            cx["init"][half] = True

                    def acc_pair(cx, src2):
                        """acc2[:, 0:2, :] (+)= src2 ([128,2,512], halves 0,1)."""
                        acc2 = cx["a"]
                        dst = acc2[:, 0:2, :]
                        if cx["init"][0] and cx["init"][1]:
                            nc.vector.tensor_add(dst, dst, src2)
                        else:
                            assert not cx["init"][0] and not cx["init"][1]
                            nc.vector.tensor_copy(dst, src2)
                            cx["init"][0] = cx["init"][1] = True

                    def pv(cx, vtile, rhs, lo=0):
                        o_ps = cx["o"]
                        cx["left"] -= 1
                        nc.tensor.matmul(
                            o_ps[:, lo:] if lo else o_ps,
                            lhsT=vtile, rhs=rhs,
                            start=cx["pv_first"], stop=(cx["left"] == 0),
                            skip_group_check=True,
                        )
                        cx["pv_first"] = False

                    def close_blk(b_, h_, blk, cx):
                        oT_sb = outp.tile([128, QBLK], BF16, tag="oT_sb")
                        nc.vector.tensor_copy(oT_sb, cx["o"])
                        nc.sync.dma_start(out=oh[h_, b_, blk], in_=oT_sb)
                        nc.sync.dma_start(out=ah[h_, b_, blk], in_=cx["a"])
                        del ctx[blk]

                    def emit_rest(u, b_, h_):
                        kind, blk, segs = groups[u]
                        pT = pbuf.pop(u)
                        cx = get_ctx(blk)
                        if kind == "off":
                            half = cx["c"] % 2
                            assert half == 0
                            acc_pair(cx, pT)
                            cx["c"] += 2
                            for i, j in enumerate(segs):
                                pv(cx, vsb[:, j, :], pT[:, i, :])
                        else:
                            # causal masks on the first 128 columns of each
                            # segment: "da" {mi0@0, mi1@512} strided, "db"
                            # {mi3@0, mi2@128} contiguous
                            if kind == "da":
                                v01 = pT.rearrange(
                                    "p (a c) -> p a c", c=512
                                )[:, :, 0:128]
                                t2 = tri2.rearrange("p (a c) -> p a c", c=128)
                                nc.vector.tensor_mul(v01, v01, t2)
                            else:
                                v23 = pT[:, 0:256]
                                nc.vector.tensor_mul(v23, v23, tri2)
                            for mi, off, n in segs:
                                half = cx["c"] % 2
                                acc_add(cx, pT[:, off : off + n], half,
                                        lo=QBLK - n)
                                cx["c"] += 1
                                pv(cx, vsb[:, blk * NMI + mi, :],
                                   pT[:, off : off + n], lo=QBLK - n)
                            if kind == "db":
                                close_blk(b_, h_, blk, cx)

                    n_u = len(groups)
                    for u in range(n_u):
                        emit_qk(u)
                        if u >= 1:
                            emit_exp(u - 1)
                        if u >= 2:
                            emit_rest(u - 2, b, h)
                    emit_exp(n_u - 1)
                    emit_rest(n_u - 2, b, h)
                    emit_rest(n_u - 1, b, h)


def build_masks():
    """[tri | tri]: lower-triangular (inclusive) causal keep-mask for the
    first 128 columns of a diagonal segment, duplicated so one [128, 256]
    tensor serves both the strided {mi0,mi1} and contiguous {mi3,mi2}
    multiplies."""
    p = np.arange(128)[:, None]
    y = np.arange(128)[None, :]
    tri = (y >= p)
    return np.concatenate([tri, tri], axis=1).astype(ml_dtypes.bfloat16)


_CACHED = {}


def _get_program():
    if "nc" not in _CACHED:
        nc = bacc.Bacc("TRN2", target_bir_lowering=False)
        qT_d = nc.dram_tensor(
            "qTh", [HPC, B, D, SEQ_LEN], BF16, kind="ExternalInput"
        ).ap()
        kT_d = nc.dram_tensor(
            "kTh", [HPC, B, D, SEQ_LEN], BF16, kind="ExternalInput"
        ).ap()
        vh = nc.dram_tensor("vh", [T, HPC, D], F32, kind="ExternalInput").ap()
        masks = nc.dram_tensor(
            "masks", [128, 256], BF16, kind="ExternalInput"
        ).ap()
        oh = nc.dram_tensor(
            "oh", [HPC, B, SEQ_LEN // QBLK, D, QBLK], BF16,
            kind="ExternalOutput",
        ).ap()
        ah = nc.dram_tensor(
            "ah", [HPC, B, SEQ_LEN // QBLK, 128, 2, QBLK], BF16,
            kind="ExternalOutput",
        ).ap()
        build_attention(nc, qT_d, kT_d, vh, masks, oh, ah, SEQ_LEN, B, HPC)
        nc.compile()  # bacc passes: split >1-wait syncs into event semaphores
        _CACHED["nc"] = nc
    return _CACHED["nc"]


def _host_resolve_kv(k, v, k_cache, v_cache, slot_mapping):
    """Apply the cache scatter+gather on the host iff it is not the identity."""
    sm = np.asarray(slot_mapping)
    if sm.shape == (T,) and np.array_equal(sm, np.arange(T, dtype=sm.dtype)):
        return k, v
    kc = np.array(k_cache, dtype=np.float32, copy=True)
    vc = np.array(v_cache, dtype=np.float32, copy=True)
    valid = sm >= 0
    kc[sm[valid]] = k.reshape(T, H * D)[valid]
    vc[sm[valid]] = v.reshape(T, H * D)[valid]
    return kc[:T].reshape(T, H, D), vc[:T].reshape(T, H, D)


def _dmajor(x):
    """[T, H, D] fp32 -> [H, B, D, S] bf16 (d-major per sequence)."""
    xb = x.astype(ml_dtypes.bfloat16)
    return np.ascontiguousarray(
        xb.reshape(B, SEQ_LEN, H, D).transpose(2, 0, 3, 1)
    )


def kernel(q, k, v, k_cache, v_cache, slot_mapping, seq_len, _trace=False,
           _trace_kwargs=None):
    q = np.asarray(q, dtype=np.float32)
    k = np.asarray(k, dtype=np.float32)
    v = np.asarray(v, dtype=np.float32)
    assert q.shape == (T, H, D), q.shape
    assert int(seq_len) == SEQ_LEN, seq_len

    k, v = _host_resolve_kv(k, v, np.asarray(k_cache), np.asarray(v_cache),
                            slot_mapping)

    qTm = _dmajor(q)  # [H, B, D, S] bf16
    kTm = _dmajor(k)
    masks = build_masks()
    nc = _get_program()
    in_maps = []
    for c in range(N_CORES):
        hs = slice(c * HPC, (c + 1) * HPC)
        in_maps.append({
            "qTh": np.ascontiguousarray(qTm[hs]),
            "kTh": np.ascontiguousarray(kTm[hs]),
            "vh": np.ascontiguousarray(v[:, hs, :]),
            "masks": masks,
        })
    res = run_bass_kernel_spmd(
        nc, in_maps, core_ids=list(range(N_CORES)),
        trace=_trace, **(_trace_kwargs or {}),
    )
    out = np.empty((T, H, D), dtype=np.float32)
    for c in range(N_CORES):
        oT = np.asarray(res.results[c]["oh"]).astype(np.float32)
        av = np.asarray(res.results[c]["ah"]).astype(np.float32)
        denom = av.sum(axis=(3, 4))  # [HPC, B, NBLK, QBLK]
        o = oT / denom[:, :, :, None, :]
        # -> [B, NBLK, QBLK, HPC, D] -> [T, HPC, D]
        o = o.transpose(1, 2, 4, 0, 3).reshape(T, HPC, D)
        out[:, c * HPC : (c + 1) * HPC, :] = o
    if _trace:
        kernel.last_results = res
    return out


# revision 10
# speedup vs baseline: 1.1585x; 1.0562x over previous
"""Trainium2 Bass kernel: fused store_kvcache + causal prefill attention.

Problem (hardcoded): T=8192 tokens, H=16 heads, D=128, seq_len=2048 (B=4
packed sequences), fp32 in/out. slot_mapping is arange(T) (contiguous slots),
so the KV-cache scatter followed by the cache gather is an identity
permutation on [0,T): attention reads exactly k/v. For robustness, any
non-identity slot_mapping is materialized on the host before the device call.

Sharding: tensor-parallel over heads. 16 heads / 8 NeuronCores = 2 heads per
core; each core runs the same Bass program on its own head slice (SPMD).
Host-side prep per core: slice the 2 heads and lay Q/K out d-major
([head, batch, d, token]) in bf16 - the layout the PE contraction needs.

Per (batch, head) the device computes, per 512-query block (bf16 matmul
operands, fp32 PSUM accumulation), with the scalar engine (exp at 1
elem/cycle/lane) as the pacing engine:
  - off-diagonal k-tiles in pairs per PSUM tile [128, 2, 512]; one QK
    matmul per k-tile, ONE exp per pair.
  - the 4 diagonal k-tiles of each query block are exact-causal in two
    chunks [mi0 512|mi1 384] and [mi3 128|mi2 256] (segment offsets keep
    every matmul output inside a single 2KB PSUM bank); masking (DVE
    multiply) only touches the triangle columns.
  - all s-tiles are <= 2 banks with a 3-deep pool, so the PE runs 2+
    groups ahead of the ACT stream and every ACTIVATE's semaphore wait is
    pre-satisfied (keeps the ~290ns instruction setup overlapped).
  - softmax denominator partials accumulate into two bf16 halves on the DVE
    (copy-init on first touch, [128,2,512] paired adds when possible); the
    host does the final 256-way sum + divide while gathering/transposing.
  - PV accumulates O^T into a 1-bank fp32 PSUM tile per query block
    (per-element has_written accumulation, exact-causal moving ranges).
PSUM budget: 3 x 2-bank score tiles + 2 x 1-bank O^T tiles = 8 banks.
"""

import numpy as np
import ml_dtypes

import concourse.bacc as bacc
import concourse.tile as tile
from concourse import mybir
from concourse.bass_utils import run_bass_kernel_spmd

# Problem constants (match the grading harness inputs).
T, H, D = 8192, 16, 128
SEQ_LEN = 2048
NUM_SLOTS = 16384
SCALE = 0.08838834764831845  # 1/sqrt(128)
N_CORES = 8
HPC = H // N_CORES  # heads per core
B = T // SEQ_LEN

BF16 = mybir.dt.bfloat16
F32 = mybir.dt.float32

QBLK = 512           # query block (one PSUM bank of fp32 for O^T)
NMI = QBLK // 128    # 128-chunks per query block

# Diagonal tiles are processed exact-causal in two 2-bank-or-less chunks:
#   chunk "da": [mi0 N=512 | mi1 N=384]  (896 elems; mi1 starts at elem 512
#               = the 2KB PSUM bank boundary, so neither matmul output
#               crosses a bank)
#   chunk "db": [mi3 N=128 | mi2 N=256]  (384 elems, one bank)
# Each chunk's causal triangle sits in the first 128 columns of each
# segment; "da" masks {0, 512} (strided), "db" masks [0:256] (contiguous).
DA_SEGS = [(0, 0, 512), (1, 512, 384)]
DB_SEGS = [(3, 0, 128), (2, 128, 256)]


def _build_groups(nblk):
    """Per-(b,h) ACT-group schedule, one ACTIVATE per group, all s-tiles
    <= 2 PSUM banks (x3 pool bufs keeps the PE 2+ groups ahead of the
    scalar engine, preserving ACT instruction-setup overlap).
      ("off", blk, [j, j+1]): two off-diagonal k-tiles of query block blk.
      ("da"|"db", blk, segs): diagonal chunks (exact causal ranges).
    The first group touching blk is full-width; diag comes last."""
    assert nblk == 4
    g = [("da", 0, DA_SEGS), ("db", 0, DB_SEGS)]
    for blk in range(1, nblk):
        for j in range(0, 4 * blk, 2):
            g.append(("off", blk, [j, j + 1]))
        g.append(("da", blk, DA_SEGS))
        g.append(("db", blk, DB_SEGS))
    return g


# PV matmuls per query block: off-diag tiles + 4 diag segs
SEGS_PER_BLK = [4, 8, 12, 16]


def build_attention(nc, qT_d, kT_d, vh, masks, oh, ah, S, B_, HPC_):
    """Emit the Tile program.

    qT_d/kT_d: DRAM APs [HPC_, B_, 128, S] bf16 (d-major Q/K).
    vh:        DRAM AP [B_*S, HPC_, 128] fp32 (natural V).
    masks:     DRAM AP [128, 256] bf16 ([tri | tri] causal masks).
    oh:        DRAM AP [HPC_, B_, NBLK, 128, QBLK] bf16 output: UNNORMALIZED
               O^T blocks (host divides by denominators and transposes back).
    ah:        DRAM AP [HPC_, B_, NBLK, 128, 2, QBLK] bf16 output: softmax
               denominator accumulator halves (host sums across the 128x2).
    """
    NT = S // 128           # 128-token tiles per sequence
    NBLK = S // QBLK        # query blocks per sequence

    with tile.TileContext(nc) as tc:
        with (
            tc.tile_pool(name="singles", bufs=1) as singles,
            tc.tile_pool(name="dmaj", bufs=2) as dmaj,
            tc.tile_pool(name="ptp", bufs=4) as ptp,
            tc.tile_pool(name="accp", bufs=3) as accp,
            tc.tile_pool(name="outp", bufs=4) as outp,
            tc.tile_pool(name="ps_s", bufs=3, space="PSUM") as ps_s,
            tc.tile_pool(name="ps_o", bufs=2, space="PSUM") as ps_o,
        ):
            tri2 = singles.tile([128, 256], BF16)
            nc.sync.dma_start(out=tri2, in_=masks)
            # dependency-free dummy exp: hoists the ~1.3us ACT table load to
            # t=0, overlapping the input DMAs instead of the first real exp
            warm = singles.tile([128, 8], BF16)
            nc.vector.memset(warm, 0.0)
            nc.scalar.activation(
                out=warm, in_=warm,
                func=mybir.ActivationFunctionType.Exp, scale=SCALE,
            )

            for b in range(B_):
                for h in range(HPC_):
                    base = b * S
                    # d-major Q/K: straight HWDGE loads, contiguous 4KB rows.
                    # Head chunks (what diag-blk0 + the first pairs consume)
                    # land first so the pipeline starts ~1us in, not after
                    # the full 1.5MB of loads.
                    qsrc = qT_d[h, b].rearrange("d (n p) -> d n p", p=128)
                    qT = dmaj.tile([128, NT, 128], BF16, tag="qT")
                    nc.gpsimd.dma_start(out=qT[:, 0:8, :], in_=qsrc[:, 0:8, :])
                    nc.gpsimd.dma_start(out=qT[:, 8:, :], in_=qsrc[:, 8:, :])
                    ksrc = kT_d[h, b].rearrange("d (n p) -> d n p", p=128)
                    kT = dmaj.tile([128, NT, 128], BF16, tag="kT")
                    nc.gpsimd.dma_start(out=kT[:, 0:4, :], in_=ksrc[:, 0:4, :])
                    nc.gpsimd.dma_start(out=kT[:, 4:, :], in_=ksrc[:, 4:, :])
                    # natural V tiles, fp32->bf16 cast in the SWDGE datapath
                    vsrc = vh[base : base + S, h, :].rearrange(
                        "(n p) d -> p n d", p=128
                    )
                    vsb = dmaj.tile([128, NT, 128], BF16, tag="vsb")
                    nc.gpsimd.dma_start(out=vsb[:, 0:4, :], in_=vsrc[:, 0:4, :])
                    nc.gpsimd.dma_start(out=vsb[:, 4:, :], in_=vsrc[:, 4:, :])

                    groups = _build_groups(NBLK)
                    ctx = {}
                    sbuf = {}   # group idx -> PSUM score tile
                    pbuf = {}   # group idx -> SBUF bf16 P tile

                    def get_ctx(blk):
                        if blk not in ctx:
                            o_ps = ps_o.tile([128, QBLK], F32, tag="o_ps")
                            acc2 = accp.tile([128, 2, QBLK], BF16, tag="acc2")
                            ctx[blk] = {
                                "o": o_ps, "a": acc2,
                                "init": [False, False],  # per-half acc init
                                "c": 0,                  # seg counter (half toggle)
                                "pv_first": True,
                                "left": SEGS_PER_BLK[blk],
                            }
                        return ctx[blk]

                    def emit_qk(u):
                        kind, blk, segs = groups[u]
                        if kind == "off":
                            s = ps_s.tile([128, 2, 512], F32, tag="s2")
                            qmov = qT[:, blk * NMI : (blk + 1) * NMI, :]
                            for i, j in enumerate(segs):
                                nc.tensor.matmul(
                                    s[:, i, :], lhsT=kT[:, j, :], rhs=qmov,
                                    start=True, stop=True,
                                )
                        else:
                            n_tot = 896 if kind == "da" else 384
                            s = ps_s.tile([128, n_tot], F32, tag="s2")
                            for mi, off, n in segs:
                                nch = n // 128
                                qmov = qT[:, (blk + 1) * NMI - nch : (blk + 1) * NMI, :]
                                nc.tensor.matmul(
                                    s[:, off : off + n],
                                    lhsT=kT[:, blk * NMI + mi, :], rhs=qmov,
                                    start=True, stop=True,
                                )
                        sbuf[u] = s

                    def emit_exp(u):
                        kind, blk, segs = groups[u]
                        s = sbuf.pop(u)
                        if kind == "off":
                            pT = ptp.tile([128, 2, 512], BF16, tag="pT")
                            dst = pT
                        elif kind == "da":
                            # padded to 1024 so the strided mask view
                            # ("p (a c) -> p a c", c=512) is constructible;
                            # [896:1024] is never written or read
                            pT = ptp.tile([128, 1024], BF16, tag="pT")
                            dst = pT[:, 0:896]
                        else:
                            pT = ptp.tile([128, 384], BF16, tag="pT")
                            dst = pT
                        nc.scalar.activation(
                            out=dst, in_=s,
                            func=mybir.ActivationFunctionType.Exp, scale=SCALE,
                        )
                        pbuf[u] = pT

                    def acc_add(cx, src, half, lo=0):
                        """acc2[:, half, lo:] (+)= src  (copy on first touch)."""
                        acc2 = cx["a"]
                        dst = acc2[:, half, lo:]
                        if cx["init"][half]:
                            nc.vector.tensor_add(dst, dst, src)
                        else:
                            if lo:
                                nc.vector.memset(acc2[:, half, 0:lo], 0.0)
                            nc.vector.tensor_copy(dst, src)
                            cx["init"][half] = True

                    def acc_pair(cx, src2):
                        """acc2[:, 0:2, :] (+)= src2 ([128,2,512], halves 0,1)."""
                        acc2 = cx["a"]
                        dst = acc2[:, 0:2, :]
                        if cx["init"][0] and cx["init"][1]:
                            nc.vector.tensor_add(dst, dst, src2)
                        else:
                            assert not cx["init"][0] and not cx["init"][1]
                            nc.vector.tensor_copy(dst, src2)
                            cx["init"][0] = cx["init"][1] = True

                    def pv(cx, vtile, rhs, lo=0):
                        o_ps = cx["o"]
                        cx["left"] -= 1
                        nc.tensor.matmul(
                            o_ps[:, lo:] if lo else o_ps,
                            lhsT=vtile, rhs=rhs,
                            start=cx["pv_first"], stop=(cx["left"] == 0),
                            skip_group_check=True,
                        )
                        cx["pv_first"] = False

                    def close_blk(b_, h_, blk, cx):
                        oT_sb = outp.tile([128, QBLK], BF16, tag="oT_sb")
                        nc.vector.tensor_copy(oT_sb, cx["o"])
                        nc.sync.dma_start(out=oh[h_, b_, blk], in_=oT_sb)
                        nc.sync.dma_start(out=ah[h_, b_, blk], in_=cx["a"])
                        del ctx[blk]

                    def emit_rest(u, b_, h_):
                        kind, blk, segs = groups[u]
                        pT = pbuf.pop(u)
                        cx = get_ctx(blk)
                        if kind == "off":
                            half = cx["c"] % 2
                            assert half == 0
                            acc_pair(cx, pT)
                            cx["c"] += 2
                            for i, j in enumerate(segs):
                                pv(cx, vsb[:, j, :], pT[:, i, :])
                        else:
                            # causal masks on the first 128 columns of each
                            # segment: "da" {mi0@0, mi1@512} strided, "db"
                            # {mi3@0, mi2@128} contiguous
                            if kind == "da":
                                v01 = pT.rearrange(
                                    "p (a c) -> p a c", c=512
                                )[:, :, 0:128]
                                t2 = tri2.rearrange("p (a c) -> p a c", c=128)
                                nc.vector.tensor_mul(v01, v01, t2)
                            else:
                                v23 = pT[:, 0:256]
                                nc.vector.tensor_mul(v23, v23, tri2)
                            for mi, off, n in segs:
                                half = cx["c"] % 2
                                acc_add(cx, pT[:, off : off + n], half,
                                        lo=QBLK - n)
                                cx["c"] += 1
                                pv(cx, vsb[:, blk * NMI + mi, :],
                                   pT[:, off : off + n], lo=QBLK - n)
                            if kind == "db":
                                close_blk(b_, h_, blk, cx)

                    n_u = len(groups)
                    for u in range(n_u):
                        emit_qk(u)
                        if u >= 1:
                            emit_exp(u - 1)
                        if u >= 2:
                            emit_rest(u - 2, b, h)
                    emit_exp(n_u - 1)
                    emit_rest(n_u - 2, b, h)
                    emit_rest(n_u - 1, b, h)


def build_masks():
    """[tri | tri]: lower-triangular (inclusive) causal keep-mask for the
    first 128 columns of a diagonal segment, duplicated so one [128, 256]
    tensor serves both the strided {mi0,mi1} and contiguous {mi3,mi2}
    multiplies."""
    p = np.arange(128)[:, None]
    y = np.arange(128)[None, :]
    tri = (y >= p)
    return np.concatenate([tri, tri], axis=1).astype(ml_dtypes.bfloat16)


_CACHED = {}


def _get_program():
    if "nc" not in _CACHED:
        nc = bacc.Bacc("TRN2", target_bir_lowering=False)
        qT_d = nc.dram_tensor(
            "qTh", [HPC, B, D, SEQ_LEN], BF16, kind="ExternalInput"
        ).ap()
        kT_d = nc.dram_tensor(
            "kTh", [HPC, B, D, SEQ_LEN], BF16, kind="ExternalInput"
        ).ap()
        vh = nc.dram_tensor("vh", [T, HPC, D], F32, kind="ExternalInput").ap()
        masks = nc.dram_tensor(
            "masks", [128, 256], BF16, kind="ExternalInput"
        ).ap()
        oh = nc.dram_tensor(
            "oh", [HPC, B, SEQ_LEN // QBLK, D, QBLK], BF16,
            kind="ExternalOutput",
        ).ap()
        ah = nc.dram_tensor(
            "ah", [HPC, B, SEQ_LEN // QBLK, 128, 2, QBLK], BF16,
            kind="ExternalOutput",
        ).ap()
        build_attention(nc, qT_d, kT_d, vh, masks, oh, ah, SEQ_LEN, B, HPC)
        nc.compile()  # bacc passes: split >1-wait syncs into event semaphores
        _CACHED["nc"] = nc
    return _CACHED["nc"]


def _host_resolve_kv(k, v, k_cache, v_cache, slot_mapping):
    """Apply the cache scatter+gather on the host iff it is not the identity."""
    sm = np.asarray(slot_mapping)
    if sm.shape == (T,) and np.array_equal(sm, np.arange(T, dtype=sm.dtype)):
        return k, v
    kc = np.array(k_cache, dtype=np.float32, copy=True)
    vc = np.array(v_cache, dtype=np.float32, copy=True)
    valid = sm >= 0
    kc[sm[valid]] = k.reshape(T, H * D)[valid]
    vc[sm[valid]] = v.reshape(T, H * D)[valid]
    return kc[:T].reshape(T, H, D), vc[:T].reshape(T, H, D)


def _dmajor(x):
    """[T, H, D] fp32 -> [H, B, D, S] bf16 (d-major per sequence)."""
    xb = x.astype(ml_dtypes.bfloat16)
    return np.ascontiguousarray(
        xb.reshape(B, SEQ_LEN, H, D).transpose(2, 0, 3, 1)
    )


def kernel(q, k, v, k_cache, v_cache, slot_mapping, seq_len, _trace=False,
           _trace_kwargs=None):
    q = np.asarray(q, dtype=np.float32)
    k = np.asarray(k, dtype=np.float32)
    v = np.asarray(v, dtype=np.float32)
    assert q.shape == (T, H, D), q.shape
    assert int(seq_len) == SEQ_LEN, seq_len

    k, v = _host_resolve_kv(k, v, np.asarray(k_cache), np.asarray(v_cache),
                            slot_mapping)

    qTm = _dmajor(q)  # [H, B, D, S] bf16
    kTm = _dmajor(k)
    masks = build_masks()
    nc = _get_program()
    in_maps = []
    for c in range(N_CORES):
        hs = slice(c * HPC, (c + 1) * HPC)
        in_maps.append({
            "qTh": np.ascontiguousarray(qTm[hs]),
            "kTh": np.ascontiguousarray(kTm[hs]),
            "vh": np.ascontiguousarray(v[:, hs, :]),
            "masks": masks,
        })
    res = run_bass_kernel_spmd(
        nc, in_maps, core_ids=list(range(N_CORES)),
        trace=_trace, **(_trace_kwargs or {}),
    )
    out = np.empty((T, H, D), dtype=np.float32)
    for c in range(N_CORES):
        oT = np.asarray(res.results[c]["oh"]).astype(np.float32)
        av = np.asarray(res.results[c]["ah"]).astype(np.float32)
        denom = av.sum(axis=(3, 4))  # [HPC, B, NBLK, QBLK]
        o = oT / denom[:, :, :, None, :]
        # -> [B, NBLK, QBLK, HPC, D] -> [T, HPC, D]
        o = o.transpose(1, 2, 4, 0, 3).reshape(T, HPC, D)
        out[:, c * HPC : (c + 1) * HPC, :] = o
    if _trace:
        kernel.last_results = res
    return out


# revision 16
# speedup vs baseline: 1.1770x; 1.0160x over previous
"""Trainium2 Bass kernel: fused store_kvcache + causal prefill attention.

Problem (hardcoded): T=8192 tokens, H=16 heads, D=128, seq_len=2048 (B=4
packed sequences), fp32 in/out. slot_mapping is arange(T) (contiguous slots),
so the KV-cache scatter followed by the cache gather is an identity
permutation on [0,T): attention reads exactly k/v. For robustness, any
non-identity slot_mapping is materialized on the host before the device call.

Sharding: tensor-parallel over heads. 16 heads / 8 NeuronCores = 2 heads per
core; each core runs the same Bass program on its own head slice (SPMD).
Host-side prep per core: slice the 2 heads and lay Q/K out d-major
([head, batch, d, token]) in bf16 - the layout the PE contraction needs.

Per (batch, head) the device computes, per 512-query block (bf16 matmul
operands, fp32 PSUM accumulation), with the scalar engine (exp at 1
elem/cycle/lane) as the pacing engine:
  - off-diagonal k-tiles in pairs per PSUM tile [128, 2, 512]; one QK
    matmul per k-tile, ONE exp per pair.
  - the 4 diagonal k-tiles of each query block are exact-causal in two
    chunks [mi0 512|mi1 384] and [mi3 128|mi2 256] (segment offsets keep
    every matmul output inside a single 2KB PSUM bank); masking (DVE
    multiply) only touches the triangle columns.
  - all s-tiles are <= 2 banks with a 3-deep pool, so the PE runs 2+
    groups ahead of the ACT stream and every ACTIVATE's semaphore wait is
    pre-satisfied (keeps the ~290ns instruction setup overlapped).
  - softmax denominator partials accumulate into two bf16 halves on the DVE
    (copy-init on first touch, [128,2,512] paired adds when possible); the
    host does the final 256-way sum + divide while gathering/transposing.
  - PV accumulates O^T into a 1-bank fp32 PSUM tile per query block
    (per-element has_written accumulation, exact-causal moving ranges).
PSUM budget: 3 x 2-bank score tiles + 2 x 1-bank O^T tiles = 8 banks.
"""

import numpy as np
import ml_dtypes

import concourse.bacc as bacc
import concourse.tile as tile
from concourse import mybir
from concourse.bass_utils import run_bass_kernel_spmd

# Problem constants (match the grading harness inputs).
T, H, D = 8192, 16, 128
SEQ_LEN = 2048
NUM_SLOTS = 16384
SCALE = 0.08838834764831845  # 1/sqrt(128)
N_CORES = 8
HPC = H // N_CORES  # heads per core
B = T // SEQ_LEN

BF16 = mybir.dt.bfloat16
F32 = mybir.dt.float32

QBLK = 512           # query block (one PSUM bank of fp32 for O^T)
NMI = QBLK // 128    # 128-chunks per query block

# Per-(b,h) schedule: 18 groups, one ACTIVATE each, every score tile
# <= 2 PSUM banks (1024 fp32). Group kinds:
#   "da"  [mi0 N=512 | mi1 N=384 @elem512]: first two diagonal tiles of a
#         query block, exact causal; triangle masks at columns {0, 512}
#         (strided view over a 1024-padded pT).
#   "dbx" [off-tile N=512 | mi3 N=128 @512 | mi2 N=256 @640]: the last two
#         diagonal tiles ride with one off-diagonal tile of the NEXT query
#         block (or the same block's last tile, for blk3); triangle masks
#         at columns [512:768] (contiguous). Closes its diag block.
#   "off" two full off-diagonal tiles (possibly from two query blocks).
# Segment elem offsets always land matmul outputs inside single 2KB banks.
# Each group seg: (blk, ktile, elem_off, N, q_lo).


def _build_groups(nblk):
    assert nblk == 4
    NMI_ = 4

    def da(b):
        return ("da", [(b, 4 * b, 0, 512, 0),
                       (b, 4 * b + 1, 512, 384, 128)])

    def dbx(b, ob, oj):
        # off-tile (ob, oj) first, then diag mi3, mi2 of block b
        return ("dbx", [(ob, oj, 0, 512, 0),
                        (b, 4 * b + 3, 512, 128, 384),
                        (b, 4 * b + 2, 640, 256, 256)])

    def off(s1, s2):
        return ("off", [(s1[0], s1[1], 0, 512, 0),
                        (s2[0], s2[1], 512, 512, 0)])

    g = [
        da(0), dbx(0, 1, 0),
        off((1, 1), (1, 2)), off((1, 3), (2, 0)),
        da(1), dbx(1, 2, 1),
        off((2, 2), (2, 3)), off((2, 4), (2, 5)), off((2, 6), (2, 7)),
        da(2), dbx(2, 3, 0),
        off((3, 1), (3, 2)), off((3, 3), (3, 4)), off((3, 5), (3, 6)),
        off((3, 7), (3, 8)), off((3, 9), (3, 10)),
        da(3), dbx(3, 3, 11),
    ]
    return g


# PV matmuls per query block: off-diag tiles + 4 diag segs
SEGS_PER_BLK = [4, 8, 12, 16]


def build_attention(nc, qT_d, kT_d, vh, masks, oh, ah, S, B_, HPC_):
    """Emit the Tile program.

    qT_d/kT_d: DRAM APs [HPC_, B_, 128, S] bf16 (d-major Q/K).
    vh:        DRAM AP [B_*S, HPC_, 128] fp32 (natural V).
    masks:     DRAM AP [128, 256] bf16 ([tri | tri] causal masks).
    oh:        DRAM AP [HPC_, B_, NBLK, 128, QBLK] bf16 output: UNNORMALIZED
               O^T blocks (host divides by denominators and transposes back).
    ah:        DRAM AP [HPC_, B_, NBLK, 128, 2, QBLK] bf16 output: softmax
               denominator accumulator halves (host sums across the 128x2).
    """
    NT = S // 128           # 128-token tiles per sequence
    NBLK = S // QBLK        # query blocks per sequence

    with tile.TileContext(nc) as tc:
        with (
            tc.tile_pool(name="singles", bufs=1) as singles,
            tc.tile_pool(name="dmaj", bufs=2) as dmaj,
            tc.tile_pool(name="ptp", bufs=4) as ptp,
            tc.tile_pool(name="accp", bufs=3) as accp,
            tc.tile_pool(name="outp", bufs=4) as outp,
            tc.tile_pool(name="ps_s", bufs=3, space="PSUM") as ps_s,
            tc.tile_pool(name="ps_o", bufs=2, space="PSUM") as ps_o,
        ):
            tri2 = singles.tile([128, 256], BF16)
            nc.sync.dma_start(out=tri2, in_=masks)
            # dependency-free dummy exp: hoists the ~1.3us ACT table load to
            # t=0, overlapping the input DMAs instead of the first real exp
            warm = singles.tile([128, 8], BF16)
            nc.vector.memset(warm, 0.0)
            nc.scalar.activation(
                out=warm, in_=warm,
                func=mybir.ActivationFunctionType.Exp, scale=SCALE,
            )

            for b in range(B_):
                for h in range(HPC_):
                    base = b * S
                    # d-major Q/K: straight HWDGE loads, contiguous 4KB rows.
                    # Head chunks (what diag-blk0 + the first pairs consume)
                    # land first so the pipeline starts ~1us in, not after
                    # the full 1.5MB of loads.
                    first = b == 0 and h == 0
                    qsrc = qT_d[h, b].rearrange("d (n p) -> d n p", p=128)
                    qT = dmaj.tile([128, NT, 128], BF16, tag="qT")
                    ksrc = kT_d[h, b].rearrange("d (n p) -> d n p", p=128)
                    kT = dmaj.tile([128, NT, 128], BF16, tag="kT")
                    if first:
                        # HWDGE head chunks: low-latency start for the very
                        # first QK (SWDGE descriptor-gen costs ~1us apiece)
                        nc.sync.dma_start(out=kT[:, 0:4, :],
                                          in_=ksrc[:, 0:4, :])
                        nc.sync.dma_start(out=qT[:, 0:8, :],
                                          in_=qsrc[:, 0:8, :])
                        nc.sync.dma_start(out=kT[:, 4:, :], in_=ksrc[:, 4:, :])
                        nc.sync.dma_start(out=qT[:, 8:, :], in_=qsrc[:, 8:, :])
                    else:
                        nc.sync.dma_start(out=qT, in_=qsrc)
                        nc.sync.dma_start(out=kT, in_=ksrc)
                    # natural V tiles, fp32->bf16 cast in the SWDGE datapath
                    vsrc = vh[base : base + S, h, :].rearrange(
                        "(n p) d -> p n d", p=128
                    )
                    vsb = dmaj.tile([128, NT, 128], BF16, tag="vsb")
                    if first:
                        nc.gpsimd.dma_start(out=vsb[:, 0:4, :],
                                            in_=vsrc[:, 0:4, :])
                        nc.gpsimd.dma_start(out=vsb[:, 4:, :],
                                            in_=vsrc[:, 4:, :])
                    else:
                        nc.gpsimd.dma_start(out=vsb, in_=vsrc)

                    groups = _build_groups(NBLK)
                    ctx = {}
                    sbuf = {}   # group idx -> PSUM score tile
                    pbuf = {}   # group idx -> SBUF bf16 P tile

                    def get_ctx(blk):
                        if blk not in ctx:
                            o_ps = ps_o.tile([128, QBLK], F32, tag="o_ps")
                            acc2 = accp.tile([128, 2, QBLK], BF16, tag="acc2")
                            ctx[blk] = {
                                "o": o_ps, "a": acc2,
                                "init": [False, False],  # per-half acc init
                                "c": 0,                  # seg counter (half toggle)
                                "pv_first": True,
                                "left": SEGS_PER_BLK[blk],
                            }
                        return ctx[blk]

                    def emit_qk(u):
                        kind, segs = groups[u]
                        n_tot = segs[-1][2] + segs[-1][3]
                        s = ps_s.tile([128, 1024], F32, tag="s2")
                        for blk, kt, off, n, lo in segs:
                            nch = n // 128
                            qmov = qT[:, (blk + 1) * NMI - nch
                                       : (blk + 1) * NMI, :]
                            nc.tensor.matmul(
                                s[:, off : off + n],
                                lhsT=kT[:, kt, :], rhs=qmov,
                                start=True, stop=True,
                            )
                        sbuf[u] = (s, n_tot)

                    def emit_exp(u):
                        kind, segs = groups[u]
                        s, n_tot = sbuf.pop(u)
                        # padded to 1024 so the "da" strided mask view is
                        # constructible; the pad is never written or read
                        pT = ptp.tile([128, 1024], BF16, tag="pT")
                        nc.scalar.activation(
                            out=pT[:, 0:n_tot], in_=s[:, 0:n_tot],
                            func=mybir.ActivationFunctionType.Exp, scale=SCALE,
                        )
                        pbuf[u] = pT

                    def acc_add(cx, src, half, lo=0):
                        """acc2[:, half, lo:] (+)= src  (copy on first touch)."""
                        acc2 = cx["a"]
                        dst = acc2[:, half, lo:]
                        if cx["init"][half]:
                            nc.vector.tensor_add(dst, dst, src)
                        else:
                            if lo:
                                nc.vector.memset(acc2[:, half, 0:lo], 0.0)
                            nc.vector.tensor_copy(dst, src)
                            cx["init"][half] = True

                    def acc_pair(cx, src2):
                        """acc2[:, 0:2, :] (+)= src2 ([128,2,512], halves 0,1)."""
                        acc2 = cx["a"]
                        dst = acc2[:, 0:2, :]
                        if cx["init"][0] and cx["init"][1]:
                            nc.vector.tensor_add(dst, dst, src2)
                        else:
                            assert not cx["init"][0] and not cx["init"][1]
                            nc.vector.tensor_copy(dst, src2)
                            cx["init"][0] = cx["init"][1] = True

                    def pv(cx, vtile, rhs, lo=0):
                        o_ps = cx["o"]
                        cx["left"] -= 1
                        nc.tensor.matmul(
                            o_ps[:, lo:] if lo else o_ps,
                            lhsT=vtile, rhs=rhs,
                            start=cx["pv_first"], stop=(cx["left"] == 0),
                            skip_group_check=True,
                        )
                        cx["pv_first"] = False

                    def close_blk(b_, h_, blk, cx):
                        oT_sb = outp.tile([128, QBLK], BF16, tag="oT_sb")
                        nc.vector.tensor_copy(oT_sb, cx["o"])
                        nc.sync.dma_start(out=oh[h_, b_, blk], in_=oT_sb)
                        nc.sync.dma_start(out=ah[h_, b_, blk], in_=cx["a"])
                        del ctx[blk]

                    def emit_rest(u, b_, h_):
                        kind, segs = groups[u]
                        pT = pbuf.pop(u)
                        # causal triangle masks first (in-place on pT)
                        if kind == "da":
                            v01 = pT.rearrange(
                                "p (a c) -> p a c", c=512
                            )[:, :, 0:128]
                            t2 = tri2.rearrange("p (a c) -> p a c", c=128)
                            nc.vector.tensor_mul(v01, v01, t2)
                        elif kind == "dbx":
                            v23 = pT[:, 512:768]
                            nc.vector.tensor_mul(v23, v23, tri2)
                        i = 0
                        closing = None
                        while i < len(segs):
                            blk, kt, off, n, lo = segs[i]
                            cx = get_ctx(blk)
                            half = cx["c"] % 2
                            nseg = segs[i + 1] if i + 1 < len(segs) else None
                            if (half == 0 and n == 512 and lo == 0 and off == 0
                                    and nseg is not None and nseg[0] == blk
                                    and nseg[3] == 512 and nseg[4] == 0
                                    and nseg[2] == 512):
                                # two adjacent full tiles, same block, on
                                # halves (0,1): one paired DVE op
                                pv2 = pT.rearrange("p (a c) -> p a c", c=512)
                                acc_pair(cx, pv2)
                                cx["c"] += 2
                                pv(cx, vsb[:, kt, :], pT[:, 0:512])
                                pv(cx, vsb[:, nseg[1], :], pT[:, 512:1024])
                                i += 2
                                continue
                            acc_add(cx, pT[:, off : off + n], half, lo=lo)
                            cx["c"] += 1
                            pv(cx, vsb[:, kt, :], pT[:, off : off + n], lo=lo)
                            if cx["left"] == 0:
                                closing = (blk, cx)
                            i += 1
                        if closing is not None:
                            close_blk(b_, h_, closing[0], closing[1])

                    n_u = len(groups)
                    for u in range(n_u):
                        emit_qk(u)
                        if u >= 1:
                            emit_exp(u - 1)
                        if u >= 2:
                            emit_rest(u - 2, b, h)
                    emit_exp(n_u - 1)
                    emit_rest(n_u - 2, b, h)
                    emit_rest(n_u - 1, b, h)


def build_masks():
    """[tri | tri]: lower-triangular (inclusive) causal keep-mask for the
    first 128 columns of a diagonal segment, duplicated so one [128, 256]
    tensor serves both the strided {mi0,mi1} and contiguous {mi3,mi2}
    multiplies."""
    p = np.arange(128)[:, None]
    y = np.arange(128)[None, :]
    tri = (y >= p)
    return np.concatenate([tri, tri], axis=1).astype(ml_dtypes.bfloat16)


_CACHED = {}


def _get_program():
    if "nc" not in _CACHED:
        nc = bacc.Bacc("TRN2", target_bir_lowering=False)
        qT_d = nc.dram_tensor(
            "qTh", [HPC, B, D, SEQ_LEN], BF16, kind="ExternalInput"
        ).ap()
        kT_d = nc.dram_tensor(
            "kTh", [HPC, B, D, SEQ_LEN], BF16, kind="ExternalInput"
        ).ap()
        vh = nc.dram_tensor("vh", [T, HPC, D], F32, kind="ExternalInput").ap()
        masks = nc.dram_tensor(
            "masks", [128, 256], BF16, kind="ExternalInput"
        ).ap()
        oh = nc.dram_tensor(
            "oh", [HPC, B, SEQ_LEN // QBLK, D, QBLK], BF16,
            kind="ExternalOutput",
        ).ap()
        ah = nc.dram_tensor(
            "ah", [HPC, B, SEQ_LEN // QBLK, 128, 2, QBLK], BF16,
            kind="ExternalOutput",
        ).ap()
        build_attention(nc, qT_d, kT_d, vh, masks, oh, ah, SEQ_LEN, B, HPC)
        nc.compile()  # bacc passes: split >1-wait syncs into event semaphores
        _CACHED["nc"] = nc
    return _CACHED["nc"]


def _host_resolve_kv(k, v, k_cache, v_cache, slot_mapping):
    """Apply the cache scatter+gather on the host iff it is not the identity."""
    sm = np.asarray(slot_mapping)
    if sm.shape == (T,) and np.array_equal(sm, np.arange(T, dtype=sm.dtype)):
        return k, v
    kc = np.array(k_cache, dtype=np.float32, copy=True)
    vc = np.array(v_cache, dtype=np.float32, copy=True)
    valid = sm >= 0
    kc[sm[valid]] = k.reshape(T, H * D)[valid]
    vc[sm[valid]] = v.reshape(T, H * D)[valid]
    return kc[:T].reshape(T, H, D), vc[:T].reshape(T, H, D)


def _dmajor(x):
    """[T, H, D] fp32 -> [H, B, D, S] bf16 (d-major per sequence)."""
    xb = x.astype(ml_dtypes.bfloat16)
    return np.ascontiguousarray(
        xb.reshape(B, SEQ_LEN, H, D).transpose(2, 0, 3, 1)
    )


def kernel(q, k, v, k_cache, v_cache, slot_mapping, seq_len, _trace=False,
           _trace_kwargs=None):
    q = np.asarray(q, dtype=np.float32)
    k = np.asarray(k, dtype=np.float32)
    v = np.asarray(v, dtype=np.float32)
    assert q.shape == (T, H, D), q.shape
    assert int(seq_len) == SEQ_LEN, seq_len

    k, v = _host_resolve_kv(k, v, np.asarray(k_cache), np.asarray(v_cache),
                            slot_mapping)

    qTm = _dmajor(q)  # [H, B, D, S] bf16
    kTm = _dmajor(k)
    masks = build_masks()
    nc = _get_program()
    in_maps = []
    for c in range(N_CORES):
        hs = slice(c * HPC, (c + 1) * HPC)
        in_maps.append({
            "qTh": np.ascontiguousarray(qTm[hs]),
            "kTh": np.ascontiguousarray(kTm[hs]),
            "vh": np.ascontiguousarray(v[:, hs, :]),
            "masks": masks,
        })
    res = run_bass_kernel_spmd(
        nc, in_maps, core_ids=list(range(N_CORES)),
        trace=_trace, **(_trace_kwargs or {}),
    )
    out = np.empty((T, H, D), dtype=np.float32)
    for c in range(N_CORES):
        oT = np.asarray(res.results[c]["oh"]).astype(np.float32)
        av = np.asarray(res.results[c]["ah"]).astype(np.float32)
        denom = av.sum(axis=(3, 4))  # [HPC, B, NBLK, QBLK]
        o = oT / denom[:, :, :, None, :]
        # -> [B, NBLK, QBLK, HPC, D] -> [T, HPC, D]
        o = o.transpose(1, 2, 4, 0, 3).reshape(T, HPC, D)
        out[:, c * HPC : (c + 1) * HPC, :] = o
    if _trace:
        kernel.last_results = res
    return out
